# revision 39
# baseline (speedup 1.0000x reference)
"""Trainium2 Bass kernel for nn_AddDropMRR (add-drop microring resonator).

Math: both outputs are magnitudes of complex-linear maps of the two real
inputs, through = |alpha_w x + beta_w a|, drop = |ad_w x + bd_w a|, so

  through^2 = A x^2 + B xa + C a^2      A=|alpha|^2, B=2Re(alpha conj(beta)),
  drop^2    = D x^2 + E xa + F a^2      C=|beta|^2  (etc. for drop)

with all six coefficients per-wavelength functions of `wavelengths` and the
scalar params -> computed on HOST (complex128), shipped once as f16 diagonal
matmul blocks (outside the timing loop).

mode "q" (default): 3-product form for both outputs. A,C,D,F are inflated
by (1+2^-9) so the PSD quadratic stays non-negative under f16 product
rounding (discriminant = Im(alpha conj(beta))^2 >= 0), and the ACT sqrt
adds a tiny positive bias (1e-5 / 1e-7) absorbing f32 accumulation dips.

Per-core budget vs HW-probed rates: the shared-HBM floor for the 16MiB of
fp16 I/O is ~53.2us (298GB/s). DVE computes the 3 products xx/xa/aa
(1.23us per fp16 TT). PE applies the 6 per-wavelength diag MACs for 7 of
the 8 chunks (267ns per [128,512] matmul -> 44.8us busy); the LAST chunk's
MACs run on DVE instead (TS/TT chain into SBUF f16, `dve_tail=1`), which
offloads PE below the DMA pole AND breaks the loop-carried PSUM
serialization at the For_i boundary (measured ~2us/iter win). ACT does the
sqrts (PSUM halves, or SBUF for the tail chunk) and issues both output
DMAs on the scalar HWDGE queue (measured faster than gpsimd swdge).
GPSIMD is deliberately IDLE: any Pool compute serializes against DVE's
2-port fast modes (shared SBUF port, measured +25us). mode "cs"
(complete-square through, 5 PE sweeps) kept for reference — it needs a
Pool product and loses badly to the port conflict.

Device graph per chunk (128 wavelengths x 2048 batch), software-pipelined
s0 DMA-in (sync q) -> s1 products (DVE) -> s2 PE diag MACs (or DVE tail)
-> s3 ACT sqrt + out DMAs (scalar q). Sharding: wavelength dim split 8
ways across cores (fully elementwise); host transposes so wavelength lies
on SBUF partitions. Coef tables load outside the For_i timing loop.
"""
import numpy as np

B = 2048           # batch
W = 8192           # wavelengths
NCORES = 8
WSH = W // NCORES  # 1024 wavelengths per core
P = 128            # SBUF partitions
NCHUNK = WSH // P  # 8 chunks per core
N_EFF = 2.4
CIRC = 2.0 * np.pi * 1e-05
DLT = 2.0 ** -9    # PSD inflation for the 3-product form
MODE = "q"
HB = B // 2        # psum half width


def _host_prep(wavelengths, coupling_1, coupling_2, phi_1, phi_2, phi_ring,
               alpha):
    """Per-wavelength quadratic-form coefficients (complex128 host)."""
    c1 = float(np.asarray(coupling_1).reshape(-1)[0])
    c2 = float(np.asarray(coupling_2).reshape(-1)[0])
    p1 = float(np.asarray(phi_1).reshape(-1)[0])
    pr = float(np.asarray(phi_ring).reshape(-1)[0])
    al = float(np.asarray(alpha).reshape(-1)[0])
    k1c = float(np.clip(c1, 0.01, 0.99))
    k2c = float(np.clip(c2, 0.01, 0.99))
    t1 = float(np.sqrt(1.0 - k1c * k1c))
    t2 = float(np.sqrt(1.0 - k2c * k2c))
    s1 = float(np.sqrt(c1))      # unclamped, as in reference
    s = float(np.sqrt(c2))       # unclamped
    kappa = float(al * np.sqrt(1.0 - c1 * c1) * np.sqrt(1.0 - c2 * c2))

    # phi in f32 exactly as the reference computes it, then f64 trig
    wl = np.asarray(wavelengths, np.float32)
    phi32 = (np.float32(2.0 * np.pi * N_EFF) / wl) * np.float32(CIRC) \
        + np.float32(pr)
    phi = phi32.astype(np.float64)

    den = 1.0 - kappa * np.exp(1j * phi)
    ring = 1j * k1c * al * np.exp(1j * (phi + p1))     # ring one-pass factor
    alpha_t = t1 + t2 * s1 * ring / den                # through: x coef
    beta_t = (t2 * s1 * s) / den                       # through: a coef
    ad = k2c * ring                                    # |j e^{j phi2}| = 1
    bd = (k2c * s) * np.ones_like(phi)

    A = np.abs(alpha_t) ** 2
    Bv = 2.0 * np.real(alpha_t * np.conj(beta_t))
    C = np.abs(beta_t) ** 2
    D = (np.abs(ad) ** 2) * (1.0 + DLT)
    E = 2.0 * np.real(ad * np.conj(bd))
    F = (np.abs(bd) ** 2) * (1.0 + DLT)
    mu = Bv / (2.0 * A)
    nu = np.maximum(C - Bv * Bv / (4.0 * A), 0.0)

    coefs = dict(
        q=np.stack([A * (1.0 + DLT), Bv, C * (1.0 + DLT),
                    D, E, F]).astype(np.float32),
        cs=np.stack([A, nu, D, E, F]).astype(np.float32),
        mu=mu.astype(np.float32),
    )
    return coefs, dict(s=s, k2c=k2c)


def _build_graph(k2c, loop_n=1, nchunk=NCHUNK, bufs=8, taper=0, mode=MODE,
                 xx_tile=True, lead=1, allhalf=False, gsz=HB, dve_tail=1,
                 dve_pos=None, **_ignored):
    """SPMD per-core graph; see module docstring. loop_n>1 wraps the body
    in an on-device For_i loop for steady-state timing; coefficient tables
    load OUTSIDE the loop."""
    import concourse.tile as tile
    from concourse import bacc, mybir, bass

    f16 = mybir.dt.float16
    f32 = mybir.dt.float32
    AF = mybir.ActivationFunctionType
    ALU = mybir.AluOpType
    ncoef = 6 if mode == "q" else 5
    ndiag = ncoef * nchunk

    wsh = nchunk * P
    nc = bacc.Bacc("TRN2", target_bir_lowering=False, debug=False,
                   num_devices=NCORES)
    x_ext = nc.declare_dram_parameter("x_t", [wsh, B], f16, isOutput=False)
    a_ext = nc.declare_dram_parameter("a_t", [wsh, B], f16, isOutput=False)
    dg_ext = nc.declare_dram_parameter("dg_t", [P, ndiag * P], f16,
                                       isOutput=False)
    if mode == "cs":
        cf_ext = nc.declare_dram_parameter("cf_t", [P, nchunk], f32,
                                           isOutput=False)
    if dve_tail:
        cq_ext = nc.declare_dram_parameter("cq_t", [P, 6 * nchunk], f32,
                                           isOutput=False)
    o1_ext = nc.declare_dram_parameter("o1_t", [wsh, B], f16, isOutput=True)
    o2_ext = nc.declare_dram_parameter("o2_t", [wsh, B], f16, isOutput=True)

    with tile.TileContext(nc) as tc:
        with tc.tile_pool(name="cst", bufs=1) as cst, \
             tc.tile_pool(name="mio", bufs=bufs) as mio, \
             tc.tile_pool(name="mout", bufs=4) as mout, \
             tc.tile_pool(name="hyb", bufs=2) as hyb, \
             tc.tile_pool(name="psum", bufs=2048 // gsz,
                          space=bass.MemorySpace.PSUM) as psum:

            dg = cst.tile([P, ndiag * P], f16, tag="dg", name="dg")
            nc.sync.dma_start(dg[:], dg_ext[:])
            if mode == "cs":
                cf = cst.tile([P, nchunk], f32, tag="cf", name="cf")
                nc.sync.dma_start(cf[:], cf_ext[:])
            if dve_tail:
                cq = cst.tile([P, 6 * nchunk], f32, tag="cq", name="cq")
                nc.sync.dma_start(cq[:], cq_ext[:])
            b1 = cst.tile([P, 1], f32, tag="b1", name="b1")
            nc.vector.memset(b1[:], 1e-5 if mode == "q" else 0.0)
            b2 = cst.tile([P, 1], f32, tag="b2", name="b2")
            nc.vector.memset(b2[:], 1e-7)

            def DG(k, c):  # diag block of coef k, chunk c
                b = k * nchunk + c
                return dg[:, b * P:(b + 1) * P]

            def body(_iv=None):
                st = {}

                def s0(vc):
                    v_id, c, lo, hi = vc
                    rs = slice(c * P, (c + 1) * P)
                    cs = slice(lo, hi)
                    xt = mio.tile([P, B], f16, tag="xt", name="xt")
                    nc.sync.dma_start(xt[:, cs], x_ext[rs, cs])
                    at = mio.tile([P, B], f16, tag="at", name="at")
                    nc.sync.dma_start(at[:, cs], a_ext[rs, cs])
                    st[v_id] = dict(xt=xt, at=at)

                def s1(vc):
                    v_id, c, lo, hi = vc
                    cs = slice(lo, hi)
                    d = st[v_id]
                    xt, at = d["xt"], d["at"]
                    if mode == "cs":
                        # aa in its OWN tile so the (slow) Pool square only
                        # waits on the `at` load, not on DVE's reads of at
                        aa = mout.tile([P, B], f16, tag="aa", name="aa")
                        nc.gpsimd.tensor_mul(aa[:, cs], at[:, cs], at[:, cs])
                        d["aa"] = aa
                        # y1 = x + mu*a, squared in place -> z1 (PE reads
                        # z1 first, so emit its chain first)
                        z1 = mio.tile([P, B], f16, tag="z1", name="z1")
                        nc.vector.tensor_scalar(z1[:, cs], at[:, cs],
                                                cf[:, c:c + 1], None,
                                                ALU.mult)
                        nc.vector.tensor_add(z1[:, cs], z1[:, cs], xt[:, cs])
                        nc.vector.tensor_mul(z1[:, cs], z1[:, cs], z1[:, cs])
                        d["z1"] = z1
                        xa = mio.tile([P, B], f16, tag="xa", name="xa")
                        nc.vector.tensor_mul(xa[:, cs], xt[:, cs], at[:, cs])
                        d["xa"] = xa
                        nc.vector.tensor_mul(xt[:, cs], xt[:, cs], xt[:, cs])
                    elif xx_tile:
                        # xx into its own tile, emitted FIRST: PE's opening
                        # matmul (A@xx) unblocks one DVE-op earlier
                        xx = mio.tile([P, B], f16, tag="xx", name="xx")
                        nc.vector.tensor_mul(xx[:, cs], xt[:, cs], xt[:, cs])
                        d["xx"] = xx
                        xa = mio.tile([P, B], f16, tag="xa", name="xa")
                        nc.vector.tensor_mul(xa[:, cs], xt[:, cs], at[:, cs])
                        d["xa"] = xa
                        nc.vector.tensor_mul(at[:, cs], at[:, cs], at[:, cs])
                    else:
                        xa = mio.tile([P, B], f16, tag="xa", name="xa")
                        nc.vector.tensor_mul(xa[:, cs], xt[:, cs], at[:, cs])
                        d["xa"] = xa
                        nc.vector.tensor_mul(xt[:, cs], xt[:, cs], xt[:, cs])
                        nc.vector.tensor_mul(at[:, cs], at[:, cs], at[:, cs])

                def s2(vc):
                    v_id, c, lo, hi = vc
                    d = st[v_id]
                    xx = d["xx"] if xx_tile else d["xt"]
                    xa = d["xa"]
                    aa = d["aa"] if mode == "cs" else d["at"]
                    cs = slice(lo, hi)
                    dve_chunks = (range(nchunk - dve_tail, nchunk)
                                  if dve_pos is None
                                  else range(dve_pos, dve_pos + dve_tail))
                    if dve_tail and c in dve_chunks:
                        # MAC on DVE into SBUF f16; ACT sqrts from SBUF.
                        # Frees PE and breaks the PSUM loop-carry chain.
                        def CQ(k):
                            return cq[:, k * nchunk + c:k * nchunk + c + 1]

                        th = hyb.tile([P, B], f16, tag="th", name="th")
                        dh = hyb.tile([P, B], f16, tag="dh", name="dh")
                        tm = hyb.tile([P, B], f16, tag="tm", name="tm")
                        for dst, k0 in ((th, 0), (dh, 3)):
                            nc.vector.tensor_scalar(dst[:, cs], xx[:, cs],
                                                    CQ(k0), None, ALU.mult)
                            nc.vector.tensor_scalar(tm[:, cs], xa[:, cs],
                                                    CQ(k0 + 1), None,
                                                    ALU.mult)
                            nc.vector.tensor_add(dst[:, cs], dst[:, cs],
                                                 tm[:, cs])
                            nc.vector.tensor_scalar(tm[:, cs], aa[:, cs],
                                                    CQ(k0 + 2), None,
                                                    ALU.mult)
                            nc.vector.tensor_add(dst[:, cs], dst[:, cs],
                                                 tm[:, cs])
                        d["th"], d["dh"] = th, dh
                        d["groups"] = None
                        return
                    groups = [(goff, min(gsz, hi - goff))
                              for goff in range(lo, hi, gsz)]
                    tps, dps = [], []
                    d["tps"], d["dps"], d["groups"] = tps, dps, groups
                    mm = nc.tensor.matmul

                    if mode == "cs":
                        tsrc = [(0, d["z1"]), (1, aa)]
                        dsrc = [(2, xx), (3, xa), (4, aa)]
                    else:
                        tsrc = [(0, xx), (1, xa), (2, aa)]
                        dsrc = [(3, xx), (4, xa), (5, aa)]

                    # group-major: finish each PSUM accumulator quickly so
                    # its bank recycles while the next group accumulates
                    def one(dst, srcs, goff, gw):
                        for i, (k, src) in enumerate(srcs):
                            dgb = DG(k, c)
                            for j in range(0, gw, 512):
                                w = min(512, gw - j)
                                mm(dst[:, j:j + w], dgb,
                                   src[:, goff + j:goff + j + w],
                                   start=(i == 0), stop=(i == len(srcs) - 1))

                    for goff, gw in groups:
                        tp = psum.tile([P, gsz], f32, tag="tp", name="tp")
                        tps.append(tp)
                        one(tp, tsrc, goff, gw)
                        dp = psum.tile([P, gsz], f32, tag="dp", name="dp")
                        dps.append(dp)
                        one(dp, dsrc, goff, gw)

                def s3(vc):
                    v_id, c, lo, hi = vc
                    d = st.pop(v_id)
                    rs = slice(c * P, (c + 1) * P)
                    cs = slice(lo, hi)
                    o1t = mout.tile([P, B], f16, tag="o1t", name="o1t")
                    o2t = mout.tile([P, B], f16, tag="o2t", name="o2t")
                    if d["groups"] is None:
                        for h in range(lo, hi, HB):
                            hs = slice(h, min(h + HB, hi))
                            nc.scalar.activation(o1t[:, hs], d["th"][:, hs],
                                                 AF.Sqrt, bias=b1[:])
                            nc.scalar.activation(o2t[:, hs], d["dh"][:, hs],
                                                 AF.Sqrt, bias=b2[:])
                    else:
                        for h, (goff, gw) in enumerate(d["groups"]):
                            hs = slice(goff, goff + gw)
                            nc.scalar.activation(o1t[:, hs],
                                                 d["tps"][h][:, 0:gw],
                                                 AF.Sqrt, bias=b1[:])
                            nc.scalar.activation(o2t[:, hs],
                                                 d["dps"][h][:, 0:gw],
                                                 AF.Sqrt, bias=b2[:])
                    nc.scalar.dma_start(o1_ext[rs, cs], o1t[:, cs])
                    nc.scalar.dma_start(o2_ext[rs, cs], o2t[:, cs])

                stages = [s0, s1, s2, s3]
                # stage offsets: s0 leads s1 by `lead` timesteps
                offs = [0, lead, lead + 1, lead + 2]
                # first/last chunks split into column halves so the pipeline
                # fills fast and drains with a short tail
                spans = []
                for c in range(nchunk):
                    if allhalf or (taper and c in (0, nchunk - 1)):
                        spans += [(c, 0, HB), (c, HB, B)]
                    else:
                        spans.append((c, 0, B))
                vchunks = [(i, c, lo, hi)
                           for i, (c, lo, hi) in enumerate(spans)]
                nv = len(vchunks)
                for t in range(nv + offs[-1]):
                    for s in range(len(stages) - 1, -1, -1):
                        i = t - offs[s]
                        if 0 <= i < nv:
                            stages[s](vchunks[i])

            if loop_n > 1:
                with tc.For_i(0, loop_n, 1):
                    body()
            else:
                body()

    nc.compile()
    return nc


def _shard_inputs(input_signal, add_signal, coefs, s, vecs=None, mode=MODE):
    x = np.asarray(input_signal, dtype=np.float32).astype(np.float16)
    a = np.asarray(add_signal, dtype=np.float32).astype(np.float16)
    cvec = coefs[mode] if isinstance(coefs, dict) else coefs
    ncoef = cvec.shape[0]
    in_maps = []
    for i in range(NCORES):
        sl = slice(i * WSH, (i + 1) * WSH)
        dgm = np.zeros((P, ncoef * NCHUNK * P), np.float16)
        csh = cvec[:, sl].reshape(ncoef, NCHUNK, P)
        for k in range(ncoef):
            for c in range(NCHUNK):
                bk = k * NCHUNK + c
                dgm[:, bk * P:(bk + 1) * P] = np.diag(
                    csh[k, c].astype(np.float16))
        m = {
            "x_t": np.ascontiguousarray(x[:, sl].T),
            "a_t": np.ascontiguousarray(a[:, sl].T),
            "dg_t": dgm,
        }
        if mode == "cs":
            m["cf_t"] = np.ascontiguousarray(
                coefs["mu"][sl].reshape(NCHUNK, P).T).astype(np.float32)
        if isinstance(coefs, dict):
            cq = coefs["q"][:, sl].reshape(6, NCHUNK, P)
            m["cq_t"] = np.ascontiguousarray(
                np.moveaxis(cq, 2, 0).reshape(P, 6 * NCHUNK))
        in_maps.append(m)
    return in_maps


def _gather_outputs(results):
    through = np.empty((B, W), np.float32)
    drop = np.empty((B, W), np.float32)
    for i in range(NCORES):
        sl = slice(i * WSH, (i + 1) * WSH)
        through[:, sl] = results[i]["o1_t"].T.astype(np.float32)
        drop[:, sl] = results[i]["o2_t"].T.astype(np.float32)
    return through, drop


def kernel(input_signal, add_signal, wavelengths, coupling_1, coupling_2,
           phi_1, phi_2, phi_ring, alpha):
    from concourse.bass_utils import run_bass_kernel_spmd

    coefs, sc = _host_prep(wavelengths, coupling_1, coupling_2, phi_1, phi_2,
                           phi_ring, alpha)
    nc = _build_graph(sc["k2c"])
    in_maps = _shard_inputs(input_signal, add_signal, coefs, sc["s"])
    res = run_bass_kernel_spmd(nc, in_maps, core_ids=list(range(NCORES)))
    return _gather_outputs(res.results)


# revision 44
# speedup vs baseline: 1.1308x; 1.1308x over previous
"""Trainium2 Bass kernel for nn_AddDropMRR (add-drop microring resonator).

Math: both outputs are magnitudes of complex-linear maps of the two real
inputs, through = |alpha_w x + beta_w a|, drop = |ad_w x + bd_w a|, so

  through^2 = A x^2 + B xa + C a^2      A=|alpha|^2, B=2Re(alpha conj(beta)),
  drop^2    = D x^2 + E xa + F a^2      C=|beta|^2  (etc. for drop)

with all six coefficients per-wavelength functions of `wavelengths` and the
scalar params -> computed on HOST (complex128), shipped once as f16 diagonal
matmul blocks (outside the timing loop).

mode "q" (default): 3-product form for both outputs. A,C,D,F are inflated
by (1+2^-9) so the PSD quadratic stays non-negative under f16 product
rounding (discriminant = Im(alpha conj(beta))^2 >= 0), and the ACT sqrt
adds a tiny positive bias (1e-5 / 1e-7) absorbing f32 accumulation dips.

Per-core budget vs HW-probed rates: the shared-HBM floor for the 16MiB of
fp16 I/O is ~53.2us (298GB/s). DVE computes the 3 products xx/xa/aa
(1.23us per fp16 TT). PE applies the 6 per-wavelength diag MACs for 7 of
the 8 chunks (267ns per [128,512] matmul -> 44.8us busy); the LAST chunk's
MACs — plus the second half of chunk 6 (`dve_half`) — run on DVE instead
(TS/TT chain into SBUF f16, `dve_tail=1`), balancing PE (~42us) against
DVE (~42us) below the DMA pole AND breaking the loop-carried PSUM
serialization at the For_i boundary (measured ~2.5us/iter total win). ACT does the
sqrts (PSUM halves, or SBUF for the tail chunk) and issues both output
DMAs on the scalar HWDGE queue (measured faster than gpsimd swdge).
GPSIMD is deliberately IDLE: any Pool compute serializes against DVE's
2-port fast modes (shared SBUF port, measured +25us). mode "cs"
(complete-square through, 5 PE sweeps) kept for reference — it needs a
Pool product and loses badly to the port conflict.

Device graph per chunk (128 wavelengths x 2048 batch), software-pipelined
s0 DMA-in (sync q) -> s1 products (DVE) -> s2 PE diag MACs (or DVE tail)
-> s3 ACT sqrt + out DMAs (scalar q). Sharding: wavelength dim split 8
ways across cores (fully elementwise); host transposes so wavelength lies
on SBUF partitions. Coef tables load outside the For_i timing loop.
"""
import numpy as np

B = 2048           # batch
W = 8192           # wavelengths
NCORES = 8
WSH = W // NCORES  # 1024 wavelengths per core
P = 128            # SBUF partitions
NCHUNK = WSH // P  # 8 chunks per core
N_EFF = 2.4
CIRC = 2.0 * np.pi * 1e-05
DLT = 2.0 ** -9    # PSD inflation for the 3-product form
MODE = "q"
HB = B // 2        # psum half width


def _host_prep(wavelengths, coupling_1, coupling_2, phi_1, phi_2, phi_ring,
               alpha):
    """Per-wavelength quadratic-form coefficients (complex128 host)."""
    c1 = float(np.asarray(coupling_1).reshape(-1)[0])
    c2 = float(np.asarray(coupling_2).reshape(-1)[0])
    p1 = float(np.asarray(phi_1).reshape(-1)[0])
    pr = float(np.asarray(phi_ring).reshape(-1)[0])
    al = float(np.asarray(alpha).reshape(-1)[0])
    k1c = float(np.clip(c1, 0.01, 0.99))
    k2c = float(np.clip(c2, 0.01, 0.99))
    t1 = float(np.sqrt(1.0 - k1c * k1c))
    t2 = float(np.sqrt(1.0 - k2c * k2c))
    s1 = float(np.sqrt(c1))      # unclamped, as in reference
    s = float(np.sqrt(c2))       # unclamped
    kappa = float(al * np.sqrt(1.0 - c1 * c1) * np.sqrt(1.0 - c2 * c2))

    # phi in f32 exactly as the reference computes it, then f64 trig
    wl = np.asarray(wavelengths, np.float32)
    phi32 = (np.float32(2.0 * np.pi * N_EFF) / wl) * np.float32(CIRC) \
        + np.float32(pr)
    phi = phi32.astype(np.float64)

    den = 1.0 - kappa * np.exp(1j * phi)
    ring = 1j * k1c * al * np.exp(1j * (phi + p1))     # ring one-pass factor
    alpha_t = t1 + t2 * s1 * ring / den                # through: x coef
    beta_t = (t2 * s1 * s) / den                       # through: a coef
    ad = k2c * ring                                    # |j e^{j phi2}| = 1
    bd = (k2c * s) * np.ones_like(phi)

    A = np.abs(alpha_t) ** 2
    Bv = 2.0 * np.real(alpha_t * np.conj(beta_t))
    C = np.abs(beta_t) ** 2
    D = (np.abs(ad) ** 2) * (1.0 + DLT)
    E = 2.0 * np.real(ad * np.conj(bd))
    F = (np.abs(bd) ** 2) * (1.0 + DLT)
    mu = Bv / (2.0 * A)
    nu = np.maximum(C - Bv * Bv / (4.0 * A), 0.0)

    coefs = dict(
        q=np.stack([A * (1.0 + DLT), Bv, C * (1.0 + DLT),
                    D, E, F]).astype(np.float32),
        cs=np.stack([A, nu, D, E, F]).astype(np.float32),
        mu=mu.astype(np.float32),
    )
    return coefs, dict(s=s, k2c=k2c)


def _build_graph(k2c, loop_n=1, nchunk=NCHUNK, bufs=8, taper=0, mode=MODE,
                 xx_tile=True, lead=1, allhalf=False, gsz=HB, dve_tail=1,
                 dve_pos=None, dve_half=True, o2_sync=False, **_ignored):
    """SPMD per-core graph; see module docstring. loop_n>1 wraps the body
    in an on-device For_i loop for steady-state timing; coefficient tables
    load OUTSIDE the loop."""
    import concourse.tile as tile
    from concourse import bacc, mybir, bass

    f16 = mybir.dt.float16
    f32 = mybir.dt.float32
    AF = mybir.ActivationFunctionType
    ALU = mybir.AluOpType
    ncoef = 6 if mode == "q" else 5
    ndiag = ncoef * nchunk

    wsh = nchunk * P
    nc = bacc.Bacc("TRN2", target_bir_lowering=False, debug=False,
                   num_devices=NCORES)
    x_ext = nc.declare_dram_parameter("x_t", [wsh, B], f16, isOutput=False)
    a_ext = nc.declare_dram_parameter("a_t", [wsh, B], f16, isOutput=False)
    dg_ext = nc.declare_dram_parameter("dg_t", [P, ndiag * P], f16,
                                       isOutput=False)
    if mode == "cs":
        cf_ext = nc.declare_dram_parameter("cf_t", [P, nchunk], f32,
                                           isOutput=False)
    if dve_tail:
        cq_ext = nc.declare_dram_parameter("cq_t", [P, 6 * nchunk], f32,
                                           isOutput=False)
    o1_ext = nc.declare_dram_parameter("o1_t", [wsh, B], f16, isOutput=True)
    o2_ext = nc.declare_dram_parameter("o2_t", [wsh, B], f16, isOutput=True)

    with tile.TileContext(nc) as tc:
        with tc.tile_pool(name="cst", bufs=1) as cst, \
             tc.tile_pool(name="mio", bufs=bufs) as mio, \
             tc.tile_pool(name="mout", bufs=4) as mout, \
             tc.tile_pool(name="hyb", bufs=2) as hyb, \
             tc.tile_pool(name="psum", bufs=2048 // gsz,
                          space=bass.MemorySpace.PSUM) as psum:

            dg = cst.tile([P, ndiag * P], f16, tag="dg", name="dg")
            nc.sync.dma_start(dg[:], dg_ext[:])
            if mode == "cs":
                cf = cst.tile([P, nchunk], f32, tag="cf", name="cf")
                nc.sync.dma_start(cf[:], cf_ext[:])
            if dve_tail:
                cq = cst.tile([P, 6 * nchunk], f32, tag="cq", name="cq")
                nc.sync.dma_start(cq[:], cq_ext[:])
            b1 = cst.tile([P, 1], f32, tag="b1", name="b1")
            nc.vector.memset(b1[:], 1e-5 if mode == "q" else 0.0)
            b2 = cst.tile([P, 1], f32, tag="b2", name="b2")
            nc.vector.memset(b2[:], 1e-7)

            def DG(k, c):  # diag block of coef k, chunk c
                b = k * nchunk + c
                return dg[:, b * P:(b + 1) * P]

            def body(_iv=None):
                st = {}

                def s0(vc):
                    v_id, c, lo, hi = vc
                    rs = slice(c * P, (c + 1) * P)
                    cs = slice(lo, hi)
                    xt = mio.tile([P, B], f16, tag="xt", name="xt")
                    nc.sync.dma_start(xt[:, cs], x_ext[rs, cs])
                    at = mio.tile([P, B], f16, tag="at", name="at")
                    nc.sync.dma_start(at[:, cs], a_ext[rs, cs])
                    st[v_id] = dict(xt=xt, at=at)

                def s1(vc):
                    v_id, c, lo, hi = vc
                    cs = slice(lo, hi)
                    d = st[v_id]
                    xt, at = d["xt"], d["at"]
                    if mode == "cs":
                        # aa in its OWN tile so the (slow) Pool square only
                        # waits on the `at` load, not on DVE's reads of at
                        aa = mout.tile([P, B], f16, tag="aa", name="aa")
                        nc.gpsimd.tensor_mul(aa[:, cs], at[:, cs], at[:, cs])
                        d["aa"] = aa
                        # y1 = x + mu*a, squared in place -> z1 (PE reads
                        # z1 first, so emit its chain first)
                        z1 = mio.tile([P, B], f16, tag="z1", name="z1")
                        nc.vector.tensor_scalar(z1[:, cs], at[:, cs],
                                                cf[:, c:c + 1], None,
                                                ALU.mult)
                        nc.vector.tensor_add(z1[:, cs], z1[:, cs], xt[:, cs])
                        nc.vector.tensor_mul(z1[:, cs], z1[:, cs], z1[:, cs])
                        d["z1"] = z1
                        xa = mio.tile([P, B], f16, tag="xa", name="xa")
                        nc.vector.tensor_mul(xa[:, cs], xt[:, cs], at[:, cs])
                        d["xa"] = xa
                        nc.vector.tensor_mul(xt[:, cs], xt[:, cs], xt[:, cs])
                    elif xx_tile:
                        # xx into its own tile, emitted FIRST: PE's opening
                        # matmul (A@xx) unblocks one DVE-op earlier
                        xx = mio.tile([P, B], f16, tag="xx", name="xx")
                        nc.vector.tensor_mul(xx[:, cs], xt[:, cs], xt[:, cs])
                        d["xx"] = xx
                        xa = mio.tile([P, B], f16, tag="xa", name="xa")
                        nc.vector.tensor_mul(xa[:, cs], xt[:, cs], at[:, cs])
                        d["xa"] = xa
                        nc.vector.tensor_mul(at[:, cs], at[:, cs], at[:, cs])
                    else:
                        xa = mio.tile([P, B], f16, tag="xa", name="xa")
                        nc.vector.tensor_mul(xa[:, cs], xt[:, cs], at[:, cs])
                        d["xa"] = xa
                        nc.vector.tensor_mul(xt[:, cs], xt[:, cs], xt[:, cs])
                        nc.vector.tensor_mul(at[:, cs], at[:, cs], at[:, cs])

                def s2(vc):
                    v_id, c, lo, hi = vc
                    d = st[v_id]
                    xx = d["xx"] if xx_tile else d["xt"]
                    xa = d["xa"]
                    aa = d["aa"] if mode == "cs" else d["at"]
                    cs = slice(lo, hi)
                    dve_chunks = (range(nchunk - dve_tail, nchunk)
                                  if dve_pos is None
                                  else range(dve_pos, dve_pos + dve_tail))
                    half_c = nchunk - dve_tail - 1 if dve_half else -1

                    def dve_mac(mc, mlo, mhi):
                        # MAC on DVE into SBUF f16; ACT sqrts from SBUF.
                        # Frees PE and breaks the PSUM loop-carry chain.
                        ms = slice(mlo, mhi)

                        def CQ(k):
                            return cq[:, k * nchunk + mc:k * nchunk + mc + 1]

                        th = hyb.tile([P, B], f16, tag="th", name="th")
                        dh = hyb.tile([P, B], f16, tag="dh", name="dh")
                        tm = hyb.tile([P, B], f16, tag="tm", name="tm")
                        for dst, k0 in ((th, 0), (dh, 3)):
                            nc.vector.tensor_scalar(dst[:, ms], xx[:, ms],
                                                    CQ(k0), None, ALU.mult)
                            nc.vector.tensor_scalar(tm[:, ms], xa[:, ms],
                                                    CQ(k0 + 1), None,
                                                    ALU.mult)
                            nc.vector.tensor_add(dst[:, ms], dst[:, ms],
                                                 tm[:, ms])
                            nc.vector.tensor_scalar(tm[:, ms], aa[:, ms],
                                                    CQ(k0 + 2), None,
                                                    ALU.mult)
                            nc.vector.tensor_add(dst[:, ms], dst[:, ms],
                                                 tm[:, ms])
                        d["th"], d["dh"] = th, dh
                        d["hyb_span"] = (mlo, mhi)

                    if dve_tail and c in dve_chunks:
                        dve_mac(c, lo, hi)
                        d["groups"] = None
                        return
                    if c == half_c and lo == 0 and hi == B:
                        # split this chunk: PE does the first half,
                        # DVE-MAC the second
                        dve_mac(c, HB, B)
                        hi = HB
                    groups = [(goff, min(gsz, hi - goff))
                              for goff in range(lo, hi, gsz)]
                    tps, dps = [], []
                    d["tps"], d["dps"], d["groups"] = tps, dps, groups
                    mm = nc.tensor.matmul

                    if mode == "cs":
                        tsrc = [(0, d["z1"]), (1, aa)]
                        dsrc = [(2, xx), (3, xa), (4, aa)]
                    else:
                        tsrc = [(0, xx), (1, xa), (2, aa)]
                        dsrc = [(3, xx), (4, xa), (5, aa)]

                    # group-major: finish each PSUM accumulator quickly so
                    # its bank recycles while the next group accumulates
                    def one(dst, srcs, goff, gw):
                        for i, (k, src) in enumerate(srcs):
                            dgb = DG(k, c)
                            for j in range(0, gw, 512):
                                w = min(512, gw - j)
                                mm(dst[:, j:j + w], dgb,
                                   src[:, goff + j:goff + j + w],
                                   start=(i == 0), stop=(i == len(srcs) - 1))

                    for goff, gw in groups:
                        tp = psum.tile([P, gsz], f32, tag="tp", name="tp")
                        tps.append(tp)
                        one(tp, tsrc, goff, gw)
                        dp = psum.tile([P, gsz], f32, tag="dp", name="dp")
                        dps.append(dp)
                        one(dp, dsrc, goff, gw)

                def s3(vc):
                    v_id, c, lo, hi = vc
                    d = st.pop(v_id)
                    rs = slice(c * P, (c + 1) * P)
                    cs = slice(lo, hi)
                    o1t = mout.tile([P, B], f16, tag="o1t", name="o1t")
                    o2t = mout.tile([P, B], f16, tag="o2t", name="o2t")
                    if d["groups"] is not None:
                        for h, (goff, gw) in enumerate(d["groups"]):
                            hs = slice(goff, goff + gw)
                            nc.scalar.activation(o1t[:, hs],
                                                 d["tps"][h][:, 0:gw],
                                                 AF.Sqrt, bias=b1[:])
                            nc.scalar.activation(o2t[:, hs],
                                                 d["dps"][h][:, 0:gw],
                                                 AF.Sqrt, bias=b2[:])
                    if "th" in d:
                        mlo, mhi = d["hyb_span"]
                        for h in range(mlo, mhi, HB):
                            hs = slice(h, min(h + HB, mhi))
                            nc.scalar.activation(o1t[:, hs], d["th"][:, hs],
                                                 AF.Sqrt, bias=b1[:])
                            nc.scalar.activation(o2t[:, hs], d["dh"][:, hs],
                                                 AF.Sqrt, bias=b2[:])
                    o2q = nc.sync if o2_sync else nc.scalar
                    nc.scalar.dma_start(o1_ext[rs, cs], o1t[:, cs])
                    o2q.dma_start(o2_ext[rs, cs], o2t[:, cs])

                stages = [s0, s1, s2, s3]
                # stage offsets: s0 leads s1 by `lead` timesteps
                offs = [0, lead, lead + 1, lead + 2]
                # first/last chunks split into column halves so the pipeline
                # fills fast and drains with a short tail
                spans = []
                for c in range(nchunk):
                    if allhalf or (taper and c in (0, nchunk - 1)):
                        spans += [(c, 0, HB), (c, HB, B)]
                    else:
                        spans.append((c, 0, B))
                vchunks = [(i, c, lo, hi)
                           for i, (c, lo, hi) in enumerate(spans)]
                nv = len(vchunks)
                for t in range(nv + offs[-1]):
                    for s in range(len(stages) - 1, -1, -1):
                        i = t - offs[s]
                        if 0 <= i < nv:
                            stages[s](vchunks[i])

            if loop_n > 1:
                with tc.For_i(0, loop_n, 1):
                    body()
            else:
                body()

    nc.compile()
    return nc


def _shard_inputs(input_signal, add_signal, coefs, s, vecs=None, mode=MODE):
    x = np.asarray(input_signal, dtype=np.float32).astype(np.float16)
    a = np.asarray(add_signal, dtype=np.float32).astype(np.float16)
    cvec = coefs[mode] if isinstance(coefs, dict) else coefs
    ncoef = cvec.shape[0]
    in_maps = []
    for i in range(NCORES):
        sl = slice(i * WSH, (i + 1) * WSH)
        dgm = np.zeros((P, ncoef * NCHUNK * P), np.float16)
        csh = cvec[:, sl].reshape(ncoef, NCHUNK, P)
        for k in range(ncoef):
            for c in range(NCHUNK):
                bk = k * NCHUNK + c
                dgm[:, bk * P:(bk + 1) * P] = np.diag(
                    csh[k, c].astype(np.float16))
        m = {
            "x_t": np.ascontiguousarray(x[:, sl].T),
            "a_t": np.ascontiguousarray(a[:, sl].T),
            "dg_t": dgm,
        }
        if mode == "cs":
            m["cf_t"] = np.ascontiguousarray(
                coefs["mu"][sl].reshape(NCHUNK, P).T).astype(np.float32)
        if isinstance(coefs, dict):
            cq = coefs["q"][:, sl].reshape(6, NCHUNK, P)
            m["cq_t"] = np.ascontiguousarray(
                np.moveaxis(cq, 2, 0).reshape(P, 6 * NCHUNK))
        in_maps.append(m)
    return in_maps


def _gather_outputs(results):
    through = np.empty((B, W), np.float32)
    drop = np.empty((B, W), np.float32)
    for i in range(NCORES):
        sl = slice(i * WSH, (i + 1) * WSH)
        through[:, sl] = results[i]["o1_t"].T.astype(np.float32)
        drop[:, sl] = results[i]["o2_t"].T.astype(np.float32)
    return through, drop


def kernel(input_signal, add_signal, wavelengths, coupling_1, coupling_2,
           phi_1, phi_2, phi_ring, alpha):
    from concourse.bass_utils import run_bass_kernel_spmd

    coefs, sc = _host_prep(wavelengths, coupling_1, coupling_2, phi_1, phi_2,
                           phi_ring, alpha)
    nc = _build_graph(sc["k2c"])
    in_maps = _shard_inputs(input_signal, add_signal, coefs, sc["s"])
    res = run_bass_kernel_spmd(nc, in_maps, core_ids=list(range(NCORES)))
    return _gather_outputs(res.results)
